# revision 28
# baseline (speedup 1.0000x reference)
"""Multi-head attention (B=2, S=2048, D=1024, H=16, dk=64) on 8 trn2 cores.

Sharding: batch (2) x head-group (4 heads each) = 8 shards.
Core c handles batch b = c // 4, heads g = c % 4 (heads 4g..4g+3).

Host-side prep per core:
  - inputs transposed to [d, s] so the contraction dim lands on SBUF
    partitions with no on-chip transposes,
  - Wq/Wk/Wv column-sharded per head group (1/sqrt(dk) folded into Wq/bq),
  - Wo row-sharded, transposed to [c, j],
  - each core emits a transposed partial output [1024, 2048]; host sums the
    4 partials per batch (bo/4 folded into each partial) and transposes back.

On-chip dataflow (per core):
  KT[m,s], QT[m,s] = W.T @ xT          (m = head-major dim, 256)
  V[k, h, dv(+ones)]                    (natural row layout, ones col for denom)
  scoresT[k, q] = KT_h.T @ QT_h         (per head, transposed scores)
  attn_u = exp(scoresT)                 (no max subtraction; scores ~ N(0,1))
  numden[65, q] = V'_h.T @ attn_u       (rows 0-63 numerator, row 64 denom)
  attn_cat[c, q] = numden[:64] * recip(denom)   (partition-broadcast recip)
  outT[j, q] = woT.T @ attn_cat + bo/4
"""

from contextlib import ExitStack

import ml_dtypes
import numpy as np

import concourse.bass as bass
import concourse.bacc as bacc
import concourse.mybir as mybir
import concourse.tile as tile
from concourse.bass_utils import run_bass_kernel_spmd

F32 = mybir.dt.float32
F32R = mybir.dt.float32r
BF16 = mybir.dt.bfloat16

D = 1024          # d_model
S = 2048          # sequence length
HCORE = 4         # heads per core
DK = 64           # head dim
M = HCORE * DK    # 256 sharded projection width
P = 128

N_CORES = 8
ST = 512          # s-tile (free dim of projection / q-tile)
N_ST = S // ST    # 4
N_DT = D // P     # 8 contraction tiles for projections
N_KT = S // P     # 16 k-tiles for attention
N_JT = D // P     # 8 output row tiles
GRP = 2           # score slots per psum group (2 banks, double buffered)




def build_mha_tile(tc, outs, ins):
    nc = tc.nc
    xqT, xkT, xvT = ins["xqT"], ins["xkT"], ins["xvT"]
    wq, wk, wv, woT = ins["wq"], ins["wk"], ins["wv"], ins["woT"]
    bq, bk, bvb, bo4 = ins["bq"], ins["bk"], ins["bvb"], ins["bo4"]
    outT = outs["outT"]

    ctx = ExitStack()
    ec = ctx.enter_context
    const = ec(tc.tile_pool(name="const", bufs=1))
    persist = ec(tc.tile_pool(name="persist", bufs=1))
    xq_pool = ec(tc.tile_pool(name="xq", bufs=8))
    xv_pool = ec(tc.tile_pool(name="xv", bufs=2))
    au_pool = ec(tc.tile_pool(name="au", bufs=6))
    out_pool = ec(tc.tile_pool(name="outb", bufs=4))
    small = ec(tc.tile_pool(name="small", bufs=2))
    rb_pool = ec(tc.tile_pool(name="rb", bufs=2))
    proj_ps = ec(tc.tile_pool(name="proj_ps", bufs=1, space="PSUM"))
    sc_ps = ec(tc.tile_pool(name="sc_ps", bufs=2, space="PSUM"))
    av_ps = ec(tc.tile_pool(name="av_ps", bufs=3, space="PSUM"))

    # ---- constants / weights ----
    wq_sb = const.tile_from(wq)     # [128, 8, 256]
    wk_sb = const.tile_from(wk)
    wv_sb = const.tile_from(wv)
    woT_sb = const.tile_from(woT)   # [128, 2, 1024]
    bq_sb = const.tile_from(bq)     # [128, 2]
    bk_sb = const.tile_from(bk)
    bvb_sb = const.tile_from(bvb)   # [128, 256]
    bo4_sb = const.tile_from(bo4)   # [128, 8]

    # Touch const tiles once per engine so their DMA-lane waits land on
    # these warmup ops instead of adding a 2nd wait to hot ACT/DVE ops
    # (walrus allows only 1 sync wait on ACT/DVE instruction structs).
    warm = const.tile([P, 16], F32)
    nc.vector.tensor_copy(warm[:, 0:2], bq_sb)
    nc.vector.tensor_copy(warm[:, 2:4], bk_sb)
    nc.vector.tensor_copy(warm[:, 4:12], bo4_sb)
    nc.vector.tensor_copy(warm[:, 12:16], bvb_sb[:, 0:4])

    ones_sb = const.tile([1, DK], BF16)
    nc.vector.memset(ones_sb, 1.0)

    # ---- persistent activations ----
    QT_sb = persist.tile([P, 2, S], BF16)          # [p, mt, s]
    KT_sb = persist.tile([P, 2, S], BF16)
    V_sb = persist.tile([P, N_KT, HCORE, DK + 1], BF16)   # [p, kt, h, dv']
    cat_sb = persist.tile([P, 2, S], BF16)         # attn_cat [c, ct, s]

    nc.vector.memset(V_sb[:, :, :, DK], 1.0)      # ones column for denominators

    def project_qk(xT3, w_sb, b_sb, dst_sb, st):
        """dst[m, st-slice] = w.T @ xT + b  for m=256 (2 partition tiles)."""
        xt = xq_pool.tile([P, N_DT, ST], BF16, tag="xt")
        nc.sync.dma_start(xt, xT3[:, :, st * ST:(st + 1) * ST])
        for mt in range(2):
            ps = proj_ps.tile([P, ST], F32, tag="proj", name="qk_ps")
            for dt in range(N_DT):
                nc.tensor.matmul(
                    ps,
                    w_sb[:, dt, mt * P:(mt + 1) * P],
                    xt[:, dt, :],
                    start=(dt == 0), stop=(dt == N_DT - 1))
            nc.vector.tensor_scalar_add(
                dst_sb[:, mt, st * ST:(st + 1) * ST], ps, b_sb[:, mt:mt + 1])

    def project_v():
        """V[k, h, dv] = xvT[:, k].T @ wv + bv, written into V_sb rows."""
        xvT3 = xvT.rearrange("(dt p) s -> p dt s", p=P)
        for ktg in range(N_KT // 4):
            xt = xv_pool.tile([P, N_DT, 4 * P], BF16, tag="xvt")
            nc.sync.dma_start(
                xt, xvT3[:, :, ktg * 4 * P:(ktg + 1) * 4 * P])
            for kl in range(4):
                kt = ktg * 4 + kl
                ps = proj_ps.tile([P, ST], F32, tag="proj", name="v_ps")[:, :M]
                for dt in range(N_DT):
                    nc.tensor.matmul(
                        ps, xt[:, dt, kl * P:(kl + 1) * P], wv_sb[:, dt, :],
                        start=(dt == 0), stop=(dt == N_DT - 1))
                nc.vector.tensor_add(
                    out=V_sb[:, kt, :, 0:DK],
                    in0=ps.rearrange("p (h d) -> p h d", h=HCORE),
                    in1=bvb_sb.rearrange("p (h d) -> p h d", h=HCORE))

    def attention(qt):
        """scoresT -> exp -> attn@V' -> normalize into cat_sb, for all heads."""
        qs = slice(qt * ST, (qt + 1) * ST)
        for hp in range(2):                       # head pairs (0,1), (2,3)
            heads = (2 * hp, 2 * hp + 1)
            nd = {h: av_ps.tile([P, ST], F32, tag="av", name=f"av_ps_{h}")
                  for h in heads}
            all_slots = [(kt, h) for kt in range(N_KT) for h in heads]
            for g in range(len(all_slots) // GRP):    # groups of GRP slots
                sc = sc_ps.tile([P, GRP, ST], F32, tag="sc")
                au = au_pool.tile([P, GRP, ST], BF16, tag="au")
                slots = all_slots[g * GRP:(g + 1) * GRP]
                for i, (kt, h) in enumerate(slots):
                    mt, p0 = h // 2, (h % 2) * DK
                    nc.tensor.matmul(
                        sc[:, i, :],
                        KT_sb[p0:p0 + DK, mt, kt * P:(kt + 1) * P],
                        QT_sb[p0:p0 + DK, mt, qs],
                        start=True, stop=True)
                nc.scalar.activation(au, sc, mybir.ActivationFunctionType.Exp)
                for i, (kt, h) in enumerate(slots):
                    nc.tensor.matmul(
                        nd[h][:DK + 1, :],
                        V_sb[:, kt, h, :],
                        au[:, i, :],
                        start=(kt == 0), stop=(kt == N_KT - 1))
            for h in heads:
                mt, p0 = h // 2, (h % 2) * DK
                recip = small.tile([1, ST], F32, tag="recip")
                nc.vector.reciprocal(recip, nd[h][DK:DK + 1, :])
                # broadcast recip across 64 partitions via a K=1 PE matmul;
                # hi+lo bf16 split keeps ~16 mantissa bits of the fp32 recip
                rhi = small.tile([1, ST], BF16, tag="rhi")
                rlo = small.tile([1, ST], BF16, tag="rlo")
                nc.vector.tensor_copy(rhi, recip)
                nc.vector.tensor_tensor(
                    rlo, recip, rhi, mybir.AluOpType.subtract)
                rb_ps = proj_ps.tile([P, ST], F32, tag="proj",
                                     name="rb_ps")[:DK, :]
                nc.tensor.matmul(rb_ps, ones_sb, rhi, start=True, stop=False)
                nc.tensor.matmul(rb_ps, ones_sb, rlo, start=False, stop=True)
                rb = rb_pool.tile([DK, ST], F32, tag="rb")
                nc.vector.tensor_copy(rb, rb_ps)
                nc.vector.tensor_mul(
                    out=cat_sb[p0:p0 + DK, mt, qs],
                    in0=nd[h][0:DK, :], in1=rb)

    def out_proj(qt):
        qs = slice(qt * ST, (qt + 1) * ST)
        outT3 = outT.rearrange("(jt p) s -> p jt s", p=P)
        ob = out_pool.tile([P, N_JT, ST], BF16, tag="ob")
        for jt in range(N_JT):
            ps = proj_ps.tile([P, ST], F32, tag="proj", name="op_ps")
            for ct in range(2):
                nc.tensor.matmul(
                    ps,
                    woT_sb[:, ct, jt * P:(jt + 1) * P],
                    cat_sb[:, ct, qs],
                    start=(ct == 0), stop=(ct == 1))
            nc.vector.tensor_scalar_add(
                ob[:, jt, :], ps, bo4_sb[:, jt:jt + 1])
        nc.sync.dma_start(outT3[:, :, qs], ob)

    xqT3 = xqT.rearrange("(dt p) s -> p dt s", p=P)
    xkT3 = xkT.rearrange("(dt p) s -> p dt s", p=P)
    for st in range(N_ST):
        project_qk(xkT3, wk_sb, bk_sb, KT_sb, st)
    project_qk(xqT3, wq_sb, bq_sb, QT_sb, 0)
    for st in range(1, N_ST):
        project_qk(xqT3, wq_sb, bq_sb, QT_sb, st)
    project_v()
    for qt in range(N_ST):
        attention(qt)
        out_proj(qt)
    ctx.close()


def build_bass():
    nc = bacc.Bacc(trn_type="TRN2", target_bir_lowering=False, debug=False)
    ins = {
        "xqT": nc.dram_tensor("xqT", (D, S), BF16, kind="ExternalInput").ap(),
        "xkT": nc.dram_tensor("xkT", (D, S), BF16, kind="ExternalInput").ap(),
        "xvT": nc.dram_tensor("xvT", (D, S), BF16, kind="ExternalInput").ap(),
        "wq": nc.dram_tensor("wq", (P, N_DT, M), BF16, kind="ExternalInput").ap(),
        "wk": nc.dram_tensor("wk", (P, N_DT, M), BF16, kind="ExternalInput").ap(),
        "wv": nc.dram_tensor("wv", (P, N_DT, M), BF16, kind="ExternalInput").ap(),
        "woT": nc.dram_tensor("woT", (P, 2, D), BF16, kind="ExternalInput").ap(),
        "bq": nc.dram_tensor("bq", (P, 2), F32, kind="ExternalInput").ap(),
        "bk": nc.dram_tensor("bk", (P, 2), F32, kind="ExternalInput").ap(),
        "bvb": nc.dram_tensor("bvb", (P, M), F32, kind="ExternalInput").ap(),
        "bo4": nc.dram_tensor("bo4", (P, N_JT), F32, kind="ExternalInput").ap(),
    }
    outs = {
        "outT": nc.dram_tensor("outT", (D, S), BF16, kind="ExternalOutput").ap(),
    }
    with tile.TileContext(nc) as tc:
        build_mha_tile(tc, outs, ins)
    nc.compile()
    return nc


def shard_inputs(query, key, value, Wq, bq, Wk, bk, Wv, bv, Wo, bo):
    """Build the 8 per-core input maps (all host-side numpy layout prep)."""
    def prep_w(W, ms, scale=1.0):
        # [d, m] -> [p, dt, m]
        wT = (np.asarray(W)[ms, :].T * scale).astype(ml_dtypes.bfloat16)
        return np.ascontiguousarray(
            wT.reshape(N_DT, P, M).transpose(1, 0, 2))

    def prep_b(b, ms, scale=1.0):
        return np.ascontiguousarray(
            (np.asarray(b)[ms] * scale).astype(np.float32).reshape(2, P).T)

    in_maps = []
    for c in range(N_CORES):
        b_idx, g = divmod(c, N_CORES // 2)
        ms = slice(g * M, (g + 1) * M)
        woT = np.ascontiguousarray(Wo[:, ms].T.astype(np.float32))
        in_maps.append({
            "xqT": np.ascontiguousarray(query[b_idx].T.astype(ml_dtypes.bfloat16)),
            "xkT": np.ascontiguousarray(key[b_idx].T.astype(ml_dtypes.bfloat16)),
            "xvT": np.ascontiguousarray(value[b_idx].T.astype(ml_dtypes.bfloat16)),
            "wq": prep_w(Wq, ms, 1.0 / np.sqrt(DK)),
            "wk": prep_w(Wk, ms),
            "wv": prep_w(Wv, ms),
            "woT": np.ascontiguousarray(
                woT.astype(ml_dtypes.bfloat16).reshape(2, P, D).transpose(1, 0, 2)),
            "bq": prep_b(bq, ms, 1.0 / np.sqrt(DK)),
            "bk": prep_b(bk, ms),
            "bvb": np.ascontiguousarray(
                np.tile(np.asarray(bv)[ms].astype(np.float32), (P, 1))),
            "bo4": np.ascontiguousarray(
                (np.asarray(bo) / (N_CORES // 2)).astype(np.float32)
                .reshape(N_JT, P).T),
        })
    return in_maps


_NC_CACHE = None


def _get_nc():
    global _NC_CACHE
    if _NC_CACHE is None:
        _NC_CACHE = build_bass()
    return _NC_CACHE


def run(inputs, trace=False, **kw):
    """Returns (full_output, BassKernelResults)."""
    inputs = {k: np.asarray(v) for k, v in inputs.items()}
    in_maps = shard_inputs(**inputs)
    res = run_bass_kernel_spmd(
        _get_nc(), in_maps, core_ids=list(range(N_CORES)), trace=trace, **kw)
    B = 2
    out = np.zeros((B, S, D), np.float32)
    for c in range(N_CORES):
        b_idx = c // (N_CORES // 2)
        out[b_idx] += np.asarray(res.results[c]["outT"]).astype(np.float32).T
    return out, res


def kernel(**inputs):
    out, _ = run(inputs)
    return out
